# revision 55
# baseline (speedup 1.0000x reference)
"""Trainium2 Bass kernel for DiffusionOperator (polynomial graph diffusion).

result = sum_k coeffs[k] * T^k x,  T = D^-1/2 A D^-1/2 (deg by edge col/source),
coeffs = softmax(MLP(graph stats)).

Strategy (8 NeuronCores, SPMD):
  * Nodes partitioned into 8 contiguous slices of R=12500 (dest/row side).
  * Reformulation: s_0 = dis*x; a_k = A @ s_{k-1} (plain 0/1 adjacency,
    summed by destination); s_k = dis^2 * a_k; result = c0*x + (sum_k c_k s_k)/dis.
    This removes all per-edge weights: the per-edge work is a pure gather +
    one-hot matmul segment-sum; dis scaling is per-node (cheap).
  * Table rows stored STRIPED (p-major): node (m, b, p) -> row m*NB*128 + p*NB + b,
    so stage loads/stores are single contiguous 128-partition DMAs and the
    AllGather concat order matches the SBUF layout directly.
  * Per step: every core gathers s_{k-1}[col] for its edges from a replicated
    full striped table in HBM (dma_gather, 256B rows), casts pieces to bf16,
    segment-sums by dest via bf16 PE matmuls with on-chip-built bf16 one-hot
    matrices (ragged per-(quadrant, block) chunk counts), scales by dis^2 on
    ACT, writes its new slice, and an AllGather rebuilds the table.
  * The polynomial combine accumulates on the fly in SBUF (no reload phase).
  * Edge index preprocessing (sort/pad/layout) is host-side numpy; all float
    math on x flows through the device kernel.

Self-contained: hardcodes full-problem shapes; builds/compiles on first call.
"""

import math
import sys
from dataclasses import dataclass

import numpy as np

for _p in ("/opt/trn_rl_repo",):
    if _p not in sys.path:
        sys.path.insert(0, _p)

import concourse.bacc as bacc
import concourse.bass as bass
import concourse.bass_isa as bass_isa
import concourse.mybir as mybir
import concourse.tile as tile
from concourse import tile_sem_assignment as _tsa
from concourse.tile_scheduler import DMAInst as _DMAInst


def _install_queue_aware_dmasw():
    """Map Pool SWDGE DMAs to DMASW lanes by queue_num (lane = q + 4*(i%2))
    so multi-queue dma_gather passes the per-queue semaphore-lock check."""
    if getattr(_tsa.TileClockTick, "_qaware", False):
        return
    orig = _tsa.TileClockTick._assign_tick

    def patched(self, inst):
        if (
            isinstance(inst, _DMAInst)
            and inst.engine == mybir.EngineType.Pool
            and not isinstance(inst, bass_isa.UserSyncedRemoteDMADescs)
        ):
            qn = int(getattr(inst, "queue_num", 0) or 0)
            ctr = self.__dict__.setdefault("_qctr", {})
            c = ctr.get(qn, 0)
            ctr[qn] = c + 1
            self.next_sw_dma_idx = qn + 4 * (c % 2)
        return orig(self, inst)

    _tsa.TileClockTick._assign_tick = patched
    _tsa.TileClockTick._qaware = True


_install_queue_aware_dmasw()

F32 = mybir.dt.float32
BF16 = mybir.dt.bfloat16
I16 = mybir.dt.int16
AF = mybir.ActivationFunctionType
ALU = mybir.AluOpType
P = 128


@dataclass(frozen=True)
class Cfg:
    N: int          # nodes
    E: int          # edges
    C: int          # channels (64)
    H: int          # mlp hidden (32)
    K: int          # poly degree (5)
    ncores: int     # 8
    nq: int         # token stream classes = 2 halves x 2 block-parities
    piece_tok: int  # tokens per dma_gather piece (ring cap: 1024 idxs/gather)
    c_need: tuple   # nq x NB tuple-of-tuples: 64-grain cap units per (stream, block)
    gran: int = 64  # token cap granularity

    @property
    def R(self):  # rows per core
        return self.N // self.ncores

    @property
    def NB(self):  # dest blocks per core
        return math.ceil(self.R / P)

    @property
    def NBP(self):  # pair-blocks per core
        return (self.NB + 1) // 2

    @property
    def NBB(self):  # padded (even) block count for the bf16 pair table
        return 2 * self.NBP

    @property
    def tail(self):  # real rows in last block
        return self.R - (self.NB - 1) * P

    @property
    def PRC(self):  # pair rows per core
        return self.NBP * P

    @property
    def QS(self):  # pair rows per source half
        return self.ncores * self.PRC // 2

    @property
    def LQ(self):  # per-stream token lengths (ragged)
        return tuple(sum(self.c_need[s]) * self.gran for s in range(self.nq))

    @property
    def LQP(self):  # padded stream length per stream (x piece_tok)
        return tuple(
            math.ceil(l / self.piece_tok) * self.piece_tok for l in self.LQ
        )

    @property
    def n_pieces(self):  # gather pieces per stream
        return tuple(l // self.piece_tok for l in self.LQP)

    @property
    def layout(self):  # window-based column layout (see _chunk_layout_win)
        return _chunk_layout_win(self.c_need, self.nq, self.NB, self.gran)

    @property
    def nchunk(self):  # total S columns in dcols
        return sum(self.layout[2])

    @property
    def max_ncc(self):  # max S columns per block (for iota/S tiles)
        return max(self.layout[2])


def _chunk_layout_win(c_need, nq, NB, gran):
    """64-grain caps viewed in 128-token gather windows; each (group, window)
    overlap is one S column: partitions outside the group's span keep dcol=255
    (is_equal -> 0), so matmuls read full 128-partition windows with no
    partition offsets. Returns (goff, cols[b]=[(q, w, col)], ncol, coloff,
    first_w, wcol)."""
    caps = [[c_need[q][b] * gran for b in range(NB)] for q in range(nq)]
    goff = [[0] * NB for _ in range(nq)]
    for q in range(nq):
        for b in range(1, NB):
            goff[q][b] = goff[q][b - 1] + caps[q][b - 1]
    cols = []
    ncol = []
    maxw = 1
    for b in range(NB):
        lst = []
        for q in range(nq):
            g0, cap = goff[q][b], caps[q][b]
            if cap == 0:
                continue
            w0 = g0 // P
            w1 = (g0 + cap - 1) // P
            maxw = max(maxw, w1 - w0 + 1)
            for w in range(w0, w1 + 1):
                lst.append((q, w, len(lst)))
        cols.append(lst)
        ncol.append(max(len(lst), 1))
    coloff = [0] * NB
    for b in range(1, NB):
        coloff[b] = coloff[b - 1] + ncol[b - 1]
    first_w = [[goff[q][b] // P for b in range(NB)] for q in range(nq)]
    wcol = [[[0] * maxw for _ in range(NB)] for _ in range(nq)]
    for b in range(NB):
        for (q, w, c) in cols[b]:
            wcol[q][b][w - first_w[q][b]] = c
    return goff, cols, ncol, coloff, first_w, wcol


FULL = dict(N=100000, E=1600000, C=64, H=32, K=5, ncores=8, nq=4, piece_tok=1024)


def _stripe_x(x, N, C, ncores):
    """[N, C] node-major -> per-core striped [P, NB*C] arrays (pad rows 0)."""
    R = N // ncores
    NB = math.ceil(R / P)
    out = []
    for m in range(ncores):
        sl = x[m * R : (m + 1) * R]
        buf = np.zeros((NB * P, C), dtype=np.float32)
        buf[:R] = sl
        # node (b, p) -> row p*NB + b
        st = buf.reshape(NB, P, C).transpose(1, 0, 2).reshape(P, NB * C)
        out.append(np.ascontiguousarray(st))
    return out


def _unstripe_out(res, N, C, ncores):
    R = N // ncores
    NB = math.ceil(R / P)
    full = np.empty((N, C), dtype=np.float32)
    for m in range(ncores):
        st = res[m].reshape(P, NB, C).transpose(1, 0, 2).reshape(NB * P, C)
        full[m * R : (m + 1) * R] = st[:R]
    return full


def _preprocess(x, edge_index, cfg_kw):
    """Host-side index preprocessing -> per-core input maps + Cfg."""
    N, E, ncores, nq = cfg_kw["N"], cfg_kw["E"], cfg_kw["ncores"], cfg_kw["nq"]
    piece_tok = cfg_kw["piece_tok"]
    R = N // ncores
    NB = math.ceil(R / P)
    SR = NB * P
    QS = ncores * SR // nq
    row = np.asarray(edge_index[0], dtype=np.int64)
    col = np.asarray(edge_index[1], dtype=np.int64)
    deg = np.bincount(col, minlength=N).astype(np.float32)

    NBP = (NB + 1) // 2
    PRC = NBP * P
    QSP = ncores * PRC // 2
    # dest (row) mapping: core m, block b, dest-lane dl
    m = row // R
    b = (row % R) // P
    dl = (row % R) % P
    # source (col) mapping to a bf16 pair-table row: pairs are block-adjacent
    # nodes (2j*P+p, (2j+1)*P+p); pair row = mc*PRC + pc*NBP + j, parity = bc%2
    mc = col // R
    lc = col % R
    pc = lc % P
    bc = lc // P
    prow = mc * PRC + pc * NBP + (bc // 2)
    par = bc % 2
    half = prow // QSP
    strm = half * 2 + par  # stream class (gather queue)
    lidx = (prow - half * QSP).astype(np.int16)

    # group tokens by (m, strm, b); ragged per-(strm, b) caps = max over cores
    ngroups = ncores * nq * NB
    key = (m * nq + strm) * NB + b
    order = np.argsort(key, kind="stable")
    counts = np.bincount(key, minlength=ngroups).reshape(ncores, nq, NB)
    GR = 64  # cap granularity
    c_need = np.ceil(counts.max(axis=0) / GR).astype(np.int64)  # [nq, NB]
    caps = c_need * GR

    # offsets of each (q, b) group inside its quadrant stream
    goff = np.zeros((nq, NB), dtype=np.int64)
    for qq in range(nq):
        goff[qq, 1:] = np.cumsum(caps[qq][:-1])
    LQ = caps.sum(axis=1)  # per-quadrant stream length
    LQP = (np.ceil(LQ / piece_tok) * piece_tok).astype(np.int64)

    # position of each edge: per-core stream = [q][group-ragged slots]
    starts = np.zeros(ngroups, dtype=np.int64)
    np.cumsum(counts.reshape(-1)[:-1], out=starts[1:])
    rank = np.arange(E, dtype=np.int64) - np.repeat(starts, counts.reshape(-1))
    # slot within (m, q) stream
    slot = np.repeat(np.tile(goff.reshape(-1), ncores), counts.reshape(-1)) + rank

    cfg = Cfg(c_need=tuple(tuple(r) for r in c_need), **cfg_kw)

    # window-column layout (shared with the build)
    _, _, ncol_l, coloff_l, first_w_l, wcol_l = cfg.layout
    first_w_arr = np.asarray(first_w_l, dtype=np.int64)
    wcol_arr = np.asarray(wcol_l, dtype=np.int64)
    coloff_arr = np.asarray(coloff_l, dtype=np.int64)

    nchunk = cfg.nchunk
    idx_np = [np.zeros((ncores, int(LQP[qq])), dtype=np.int16) for qq in range(nq)]
    dst_all = np.full((ncores, nchunk * P), 255.0, dtype=np.float32)

    # slot/rank are indexed by sorted position; reindex edge attrs to match
    eq = strm[order]
    em = m[order]
    eb = b[order]
    es = slot
    el = lidx[order]
    ed = dl[order]
    for qq in range(nq):
        sel = eq == qq
        idx_np[qq][em[sel], es[sel]] = el[sel]
    # dcols position: column of (group, window), partition = offset in window
    ww = es // P
    col = wcol_arr[eq, eb, ww - first_w_arr[eq, eb]]
    dpos = (coloff_arr[eb] + col) * P + (es % P)
    dst_all[em, dpos] = ed.astype(np.float32)

    in_maps = []
    xs_st = _stripe_x(np.asarray(x, dtype=np.float32), N, cfg_kw["C"], ncores)
    for mm in range(ncores):
        # wrapped int16 indices: token i of stream q -> [i%16, i//16]; replicate
        # the 16-partition pattern across all 128 partitions (8 gpsimd cores).
        wr = np.concatenate(
            [
                np.tile(
                    idx_np[qq][mm].reshape(int(LQP[qq]) // 16, 16).T, (8, 1)
                )
                for qq in range(nq)
            ],
            axis=1,
        )  # [128, sum(LQP)//16]
        degp = np.ones(NB * P, dtype=np.float32)
        degp[:R] = deg[mm * R : (mm + 1) * R]
        degp = (
            degp.reshape(NB, P).T  # node (b, p) -> [p, b]
        )
        in_maps.append(
            {
                "xs": xs_st[mm],
                "degp": np.ascontiguousarray(degp),
                "dcols": np.ascontiguousarray(
                    dst_all[mm].reshape(nchunk, P).T.astype(np.float32)
                ),
                "idx": np.ascontiguousarray(wr),
            }
        )
    return cfg, in_maps


def _build_program(cfg: Cfg):
    nc = bacc.Bacc(
        "TRN2", num_swdge_queues=cfg.nq, dynamic_dma_scratch_size=32768
    )
    C, NB, K = cfg.C, cfg.NB, cfg.K
    C2 = 2 * C
    FB = NB * C       # free elements per fp32 stage row
    FBB = cfg.NBB * C  # free elements per bf16 (pair-padded) stage row
    PAIRROWS = cfg.ncores * cfg.PRC
    cn = cfg.c_need
    LQP = cfg.LQP
    n_pieces = cfg.n_pieces
    pt = cfg.piece_tok
    ptC = pt // P * C2  # free elems per gathered piece (bf16 pair rows)

    # window-column layout (shared with _preprocess)
    _goff, cols_l, ncol_l, coloff_l, _fw, _wc = cfg.layout
    idx_qoff = [0] * cfg.nq  # idx tile column offset (in int16 units /16)
    for q in range(1, cfg.nq):
        idx_qoff[q] = idx_qoff[q - 1] + LQP[q - 1] // 16

    xs_t = nc.declare_dram_parameter("xs", [P, FB], F32, isOutput=False)
    degp_t = nc.declare_dram_parameter("degp", [P, NB], F32, isOutput=False)
    dcols_t = nc.declare_dram_parameter("dcols", [P, cfg.nchunk], F32, isOutput=False)
    idx_t = nc.declare_dram_parameter(
        "idx", [P, sum(LQP) // 16], I16, isOutput=False
    )
    w1t_t = nc.declare_dram_parameter("w1t", [C + 4, cfg.H], F32, isOutput=False)
    b1c_t = nc.declare_dram_parameter("b1c", [cfg.H, 1], F32, isOutput=False)
    w2t_t = nc.declare_dram_parameter("w2t", [cfg.H, K + 1], F32, isOutput=False)
    b2r_t = nc.declare_dram_parameter("b2r", [1, K + 1], F32, isOutput=False)
    out_t = nc.declare_dram_parameter("out", [P, FB], F32, isOutput=True)

    s_loc = [nc.dram_tensor(f"s_loc{k}", [P, FBB], BF16) for k in range(K)]
    table = [
        nc.dram_tensor(f"table{k}", [PAIRROWS, C2], BF16, addr_space="Shared")
        for k in range(K)
    ]
    stats_loc = nc.dram_tensor("stats_loc", [136], F32)
    stats_red = nc.dram_tensor("stats_red", [136], F32, addr_space="Shared")

    groups = [list(range(cfg.ncores))]

    with tile.TileContext(nc) as tc:
        with (
            tc.tile_pool(name="const", bufs=1) as cpool,
            tc.tile_pool(name="stage", bufs=2) as stpool,
            tc.tile_pool(name="sb", bufs=2) as sbpool,
            tc.tile_pool(name="gp", bufs=10) as gpool,
            tc.tile_pool(name="sp", bufs=3) as spool,
            tc.tile_pool(name="small", bufs=2) as smpool,
            tc.tile_pool(name="pmain", bufs=6, space="PSUM") as pmain,
            tc.tile_pool(name="psmall", bufs=2, space="PSUM") as psmall,
        ):
            # ---- constants ----
            iota_t = cpool.tile([P, cfg.max_ncc * P], BF16)
            nc.gpsimd.iota(
                iota_t[:],
                [[0, cfg.max_ncc], [1, P]],
                channel_multiplier=0,
                allow_small_or_imprecise_dtypes=True,
            )
            ones_col = cpool.tile([P, 1], F32)
            nc.gpsimd.memset(ones_col[:], 1.0)
            ones_row = cpool.tile([1, P], F32)
            nc.gpsimd.memset(ones_row[:], 1.0)

            idxs = cpool.tile([P, sum(LQP) // 16], I16)
            nc.sync.dma_start(out=idxs[:], in_=idx_t[:])
            dcols_f = cpool.tile([P, cfg.nchunk], F32)
            nc.sync.dma_start(out=dcols_f[:], in_=dcols_t[:])
            dcols = cpool.tile([P, cfg.nchunk], BF16)
            nc.vector.tensor_copy(dcols[:], dcols_f[:])
            degp = cpool.tile([P, NB], F32)
            nc.sync.dma_start(out=degp[:], in_=degp_t[:])
            w1t = cpool.tile([C + 4, cfg.H], F32)
            nc.sync.dma_start(out=w1t[:], in_=w1t_t[:])
            b1c = cpool.tile([cfg.H, 1], F32)
            nc.sync.dma_start(out=b1c[:], in_=b1c_t[:])
            w2t = cpool.tile([cfg.H, K + 1], F32)
            nc.sync.dma_start(out=w2t[:], in_=w2t_t[:])
            b2r = cpool.tile([1, K + 1], F32)
            nc.sync.dma_start(out=b2r[:], in_=b2r_t[:])

            # dis = min(deg^-0.5, 1e6); dis2 = dis^2; rdis = 1/dis
            dis = cpool.tile([P, NB], F32)
            nc.scalar.activation(dis[:], degp[:], AF.Sqrt)
            nc.vector.tensor_scalar_max(dis[:], dis[:], 1.0e-6)
            nc.vector.reciprocal(dis[:], dis[:])
            dis2 = cpool.tile([P, NB], F32)
            nc.vector.tensor_tensor(dis2[:], dis[:], dis[:], op=ALU.mult)
            rdis = cpool.tile([P, NB], F32)
            nc.vector.reciprocal(rdis[:], dis[:])

            acc = cpool.tile([P, FB], F32)

            # ---- phase A: load x, stats partials, s0 ----
            x_t = stpool.tile([P, FB], F32, tag="stage")
            nc.sync.dma_start(out=x_t[:], in_=xs_t[:])

            # per-channel sums over this core's rows: accumulate X_b^T @ ones
            csum_ps = psmall.tile([P, C], F32, tag="sm")
            for b in range(NB):
                nc.tensor.matmul(
                    csum_ps[0:C, 0:1],
                    lhsT=x_t[:, b * C : (b + 1) * C],
                    rhs=ones_col[:],
                    start=(b == 0),
                    stop=(b == NB - 1),
                )
            csum_sb = smpool.tile([C, 1], F32)
            nc.vector.tensor_copy(csum_sb[:], csum_ps[0:C, 0:1])

            # sum of squares per channel: reduce x_t^2 over partitions via matmul
            xsq = stpool.tile([P, FB], F32, tag="stage")
            nc.vector.tensor_tensor(xsq[:], x_t[:], x_t[:], op=ALU.mult)
            sq_ps = psmall.tile([P, C], F32, tag="sm")
            for b in range(NB):
                nc.tensor.matmul(
                    sq_ps[0:C, 0:1],
                    lhsT=xsq[:, b * C : (b + 1) * C],
                    rhs=ones_col[:],
                    start=(b == 0),
                    stop=(b == NB - 1),
                )
            sqch = smpool.tile([C, 1], F32)
            nc.vector.tensor_copy(sqch[:], sq_ps[0:C, 0:1])

            zpad = smpool.tile([1, 8], F32, tag="zp")
            nc.gpsimd.memset(zpad[:], 0.0)
            nc.sync.dma_start(out=stats_loc[0:C], in_=csum_sb[:])
            nc.sync.dma_start(out=stats_loc[C : 2 * C], in_=sqch[:])
            nc.sync.dma_start(out=stats_loc[2 * C : 2 * C + 8], in_=zpad[:])
            nc.gpsimd.collective_compute(
                "AllReduce",
                ALU.add,
                replica_groups=groups,
                ins=[stats_loc[:]],
                outs=[stats_red[:]],
            )

            # s0 = dis * x  (blockwise per-partition scale on ACT, bf16 out)
            s0_b = sbpool.tile([P, FBB], BF16, tag="sb")
            if cfg.NBB > NB:
                nc.gpsimd.memset(s0_b[:, NB * C : FBB], 0.0)
            for b in range(NB):
                nc.scalar.activation(
                    s0_b[:, b * C : (b + 1) * C],
                    x_t[:, b * C : (b + 1) * C],
                    AF.Copy,
                    scale=dis[:, b : b + 1],
                )
            nc.sync.dma_start(out=s_loc[0][:], in_=s0_b[:])
            nc.gpsimd.collective_compute(
                "AllGather",
                ALU.bypass,
                replica_groups=groups,
                ins=[s_loc[0][:]],
                outs=[table[0][:]],
            )

            # ---- coeff MLP (runs concurrently with diffusion steps) ----
            red = smpool.tile([1, 136], F32)
            nc.sync.dma_start(out=red[:], in_=stats_red[:])
            cin = smpool.tile([P, 1], F32, tag="cin")
            nc.sync.dma_start(out=cin[0:C, 0:1], in_=red[0:1, 0:C])
            nc.vector.tensor_scalar_mul(cin[0:C, 0:1], cin[0:C, 0:1], 1.0 / cfg.N)
            M = float(cfg.N * cfg.C)
            mean = smpool.tile([1, 1], F32, tag="m1")
            nc.vector.tensor_reduce(
                mean[:], red[0:1, 0:C], axis=mybir.AxisListType.X, op=ALU.add
            )
            nc.scalar.mul(mean[:], mean[:], 1.0 / M)
            sqred = smpool.tile([1, 1], F32, tag="m2")
            nc.vector.tensor_reduce(
                sqred[:], red[0:1, C : 2 * C], axis=mybir.AxisListType.X, op=ALU.add
            )
            msq = smpool.tile([1, 1], F32, tag="m3")
            nc.vector.tensor_tensor(msq[:], mean[:], mean[:], op=ALU.mult)
            nc.scalar.mul(msq[:], msq[:], -M)
            nc.vector.tensor_tensor(msq[:], sqred[:], msq[:], op=ALU.add)
            nc.scalar.mul(msq[:], msq[:], 1.0 / (M - 1.0))
            nc.scalar.activation(msq[:], msq[:], AF.Sqrt)  # std
            srow = smpool.tile([1, 4], F32, tag="m4")
            nc.vector.tensor_copy(srow[0:1, 0:1], mean[:])
            nc.vector.tensor_copy(srow[0:1, 1:2], msq[:])
            nc.gpsimd.memset(srow[0:1, 2:3], float(cfg.N))
            nc.gpsimd.memset(srow[0:1, 3:4], float(cfg.E))
            nc.gpsimd.dma_start(out=cin[C : C + 4, 0:1], in_=srow[:])

            h_ps = psmall.tile([P, C], F32, tag="sm")
            nc.tensor.matmul(
                h_ps[0 : cfg.H, 0:1], lhsT=w1t[:], rhs=cin[0 : C + 4, 0:1],
                start=True, stop=True,
            )
            h_sb = smpool.tile([cfg.H, 1], F32, tag="h")
            nc.scalar.activation(h_sb[:], h_ps[0 : cfg.H, 0:1], AF.Relu, bias=b1c[:])
            c_ps = psmall.tile([P, C], F32, tag="sm")
            nc.tensor.matmul(
                c_ps[0:1, 0 : K + 1], lhsT=h_sb[:], rhs=w2t[:], start=True, stop=True
            )
            z = smpool.tile([1, K + 1], F32, tag="z")
            nc.vector.tensor_tensor(z[:], c_ps[0:1, 0 : K + 1], b2r[:], op=ALU.add)
            zmax = smpool.tile([1, 1], F32, tag="m5")
            nc.vector.tensor_reduce(zmax[:], z[:], axis=mybir.AxisListType.X, op=ALU.max)
            nc.vector.tensor_scalar(
                z[:], z[:], zmax[0:1, 0:1], None, op0=ALU.subtract
            )
            nc.scalar.activation(z[:], z[:], AF.Exp)
            zsum = smpool.tile([1, 1], F32, tag="m6")
            nc.vector.tensor_reduce(zsum[:], z[:], axis=mybir.AxisListType.X, op=ALU.add)
            nc.vector.reciprocal(zsum[:], zsum[:])
            nc.vector.tensor_scalar_mul(z[:], z[:], zsum[0:1, 0:1])
            cb_ps = psmall.tile([P, C], F32, tag="sm")
            nc.tensor.matmul(
                cb_ps[:, 0 : K + 1], lhsT=ones_row[:], rhs=z[:], start=True, stop=True
            )
            c_bc = cpool.tile([P, K + 1], F32)
            nc.vector.tensor_copy(c_bc[:], cb_ps[:, 0 : K + 1])

            # acc = c0 * x (consumes x_t before the stage pool recycles it)
            nc.vector.tensor_scalar_mul(acc[:], x_t[:], c_bc[:, 0:1])

            # ---- phase B: K diffusion steps with on-the-fly combine ----
            pt16 = pt // 16
            for k in range(1, K + 1):
                src = table[k - 1]
                piecesb = [[None] * n_pieces[q] for q in range(cfg.nq)]
                for i in range(max(n_pieces)):
                    for q in range(cfg.nq):
                        if i >= n_pieces[q]:
                            continue
                        gt = gpool.tile([P, ptC], BF16, tag="g")
                        nc.gpsimd.dma_gather(
                            gt[:].rearrange("p (c f) -> p c f", f=C2),
                            src[(q // 2) * cfg.QS : (q // 2 + 1) * cfg.QS, :],
                            idxs[:, idx_qoff[q] + i * pt16 : idx_qoff[q] + (i + 1) * pt16],
                            num_idxs=pt,
                            num_idxs_reg=pt,
                            elem_size=C2,
                            queue_num=q,
                        )
                        piecesb[q][i] = gt

                s_stk = stpool.tile([P, FB], F32, tag="stage")
                s_b16 = sbpool.tile([P, FBB], BF16, tag="sb")
                if k < K and cfg.NBB > NB:
                    nc.gpsimd.memset(s_b16[:, NB * C : FBB], 0.0)
                for b in range(NB):
                    ps = pmain.tile([P, C], F32, tag="ps")
                    nccb = ncol_l[b]
                    nch = len(cols_l[b])
                    S = spool.tile([P, cfg.max_ncc * P], BF16, tag="S")
                    nc.vector.tensor_tensor(
                        S[:, 0 : nccb * P].rearrange("p (c f) -> p c f", f=P),
                        dcols[:, coloff_l[b] : coloff_l[b] + nccb].to_broadcast(
                            [P, nccb, P]
                        ),
                        iota_t[:, 0 : nccb * P].rearrange("p (c f) -> p c f", f=P),
                        op=ALU.is_equal,
                    )
                    for ci, (q, w, col) in enumerate(cols_l[b]):
                        par = q % 2
                        off = w * P
                        gb = piecesb[q][off // pt]
                        gv = gb[:].rearrange("p (c f) -> p c f", f=C2)
                        nc.tensor.matmul(
                            ps[:],
                            lhsT=S[:, col * P : (col + 1) * P],
                            rhs=gv[:, (off % pt) // P, par * C : par * C + C],
                            start=(ci == 0),
                            stop=(ci == nch - 1),
                        )
                    # s_k = dis^2 * a_k (fp32 for accumulation; bf16 for table)
                    nc.scalar.activation(
                        s_stk[:, b * C : (b + 1) * C],
                        ps[:],
                        AF.Copy,
                        scale=dis2[:, b : b + 1],
                    )
                    if k < K:
                        nc.scalar.activation(
                            s_b16[:, b * C : (b + 1) * C],
                            ps[:],
                            AF.Copy,
                            scale=dis2[:, b : b + 1],
                        )
                if k < K:
                    nc.sync.dma_start(out=s_loc[k][:], in_=s_b16[:])
                    nc.gpsimd.collective_compute(
                        "AllGather",
                        ALU.bypass,
                        replica_groups=groups,
                        ins=[s_loc[k][:]],
                        outs=[table[k][:]],
                    )
                # acc += (c_k * rdis) * s_k  — fold the final /dis into the
                # accumulation. In-place scale of s_stk happens after the
                # s_loc DMA has read it (Tile WAR dependency).
                crd = smpool.tile([P, NB], F32, tag="crd")
                nc.vector.tensor_scalar_mul(crd[:], rdis[:], c_bc[:, k : k + 1])
                nc.vector.tensor_tensor(
                    s_stk[:].rearrange("p (b f) -> p b f", f=C),
                    s_stk[:].rearrange("p (b f) -> p b f", f=C),
                    crd[:].to_broadcast([P, NB, C]),
                    op=ALU.mult,
                )
                nc.vector.tensor_tensor(acc[:], acc[:], s_stk[:], op=ALU.add)

            nc.sync.dma_start(out=out_t[:], in_=acc[:])

    nc.finalize()
    return nc


_CACHE = {}


def _get_program(cfg: Cfg):
    if cfg not in _CACHE:
        _CACHE[cfg] = _build_program(cfg)
    return _CACHE[cfg]


def _run(inputs, trace=False, cfg_kw=None):
    from concourse.bass_utils import run_bass_kernel_spmd

    cfg_kw = dict(cfg_kw or FULL)
    x = np.asarray(inputs["x"], dtype=np.float32)
    cfg, in_maps = _preprocess(x, inputs["edge_index"], cfg_kw)
    W1 = np.asarray(inputs["W1"], dtype=np.float32)
    b1 = np.asarray(inputs["b1"], dtype=np.float32)
    W2 = np.asarray(inputs["W2"], dtype=np.float32)
    b2 = np.asarray(inputs["b2"], dtype=np.float32)
    for im in in_maps:
        im["w1t"] = np.ascontiguousarray(W1.T)
        im["b1c"] = np.ascontiguousarray(b1[:, None])
        im["w2t"] = np.ascontiguousarray(W2.T)
        im["b2r"] = np.ascontiguousarray(b2[None, :])
    nc = _get_program(cfg)
    res = run_bass_kernel_spmd(
        nc, in_maps, core_ids=list(range(cfg.ncores)), trace=trace
    )
    out = _unstripe_out(
        [res.results[i]["out"] for i in range(cfg.ncores)],
        cfg.N, cfg.C, cfg.ncores,
    )
    return out, res.exec_time_ns


def kernel(**inputs) -> np.ndarray:
    out, _ = _run(inputs)
    return out


# ---------------------------------------------------------------------------
# toy-scale validation against a numpy port of the reference, via CoreSim
# ---------------------------------------------------------------------------


def _np_reference(x, edge_index, W1, b1, W2, b2, K=5):
    N, C = x.shape
    E = edge_index.shape[1]
    row, col = edge_index[0].astype(np.int64), edge_index[1].astype(np.int64)
    deg = np.bincount(col, minlength=N).astype(np.float32)
    with np.errstate(divide="ignore"):
        dis = np.minimum(deg ** -0.5, 1e6).astype(np.float32)
    norm = dis[row] * dis[col]
    xm = x.mean(axis=0)
    stats = np.array([x.mean(), x.std(ddof=1), N, E], dtype=np.float32)
    cin = np.concatenate([xm, stats])
    h = np.maximum(W1 @ cin + b1, 0.0)
    zz = W2 @ h + b2
    zz = np.exp(zz - zz.max())
    coeffs = zz / zz.sum()
    result = coeffs[0] * x
    tx = x.copy()
    for k in range(1, K + 1):
        nt = np.zeros_like(tx)
        np.add.at(nt, row, norm[:, None] * tx[col])
        tx = nt
        result = result + coeffs[k] * tx
    return result


def _selftest_sim():
    from concourse.bass_interp import MultiCoreSim

    rng = np.random.default_rng(0)
    kw = dict(N=2400, E=9600, C=64, H=32, K=5, ncores=8, nq=4, piece_tok=384)
    x = rng.standard_normal((kw["N"], kw["C"])).astype(np.float32)
    ei = rng.integers(0, kw["N"], size=(2, kw["E"])).astype(np.int32)
    W1 = rng.uniform(-1, 1, (kw["H"], kw["C"] + 4)).astype(np.float32) / 8
    b1 = rng.uniform(-1, 1, (kw["H"],)).astype(np.float32) / 8
    W2 = rng.uniform(-1, 1, (kw["K"] + 1, kw["H"])).astype(np.float32) / 5
    b2 = rng.uniform(-1, 1, (kw["K"] + 1,)).astype(np.float32) / 5

    cfg, in_maps = _preprocess(x, ei, kw)
    print("toy cfg: NB", cfg.NB, "nchunk", cfg.nchunk, "LQ", cfg.LQ,
          "n_pieces", cfg.n_pieces, "max_ncc", cfg.max_ncc)
    for im in in_maps:
        im["w1t"] = np.ascontiguousarray(W1.T)
        im["b1c"] = np.ascontiguousarray(b1[:, None])
        im["w2t"] = np.ascontiguousarray(W2.T)
        im["b2r"] = np.ascontiguousarray(b2[None, :])
    nc = _build_program(cfg)
    sim = MultiCoreSim(nc, cfg.ncores)
    for i in range(cfg.ncores):
        for name, arr in in_maps[i].items():
            sim.cores[i].tensor(name)[:] = arr
    sim.simulate()
    out = _unstripe_out(
        [np.array(sim.cores[i].tensor("out")) for i in range(cfg.ncores)],
        kw["N"], kw["C"], kw["ncores"],
    )
    exp = _np_reference(x, ei, W1, b1, W2, b2, K=kw["K"])
    err = np.abs(out - exp).max() / (np.abs(exp).max() + 1e-30)
    rel = np.linalg.norm(out - exp) / (np.linalg.norm(exp) + 1e-30)
    print(f"sim selftest: max-abs-rel {err:.3e}  fro-rel {rel:.3e}")
    assert rel < 2e-2, (rel, err)
    print("SIM SELFTEST PASSED")


if __name__ == "__main__":
    _selftest_sim()
